# revision 45
# baseline (speedup 1.0000x reference)
"""MultiHeadAttention TRN2 kernel: B=2, S=2048, D=1024, H=16, DK=64, 8 cores.

Sharding: core c handles batch b=c//4 and heads hg=(c%4)*4 .. +3 (data + head
parallel). Projections column-split by head; out-proj row-split; the
all-reduce after out-proj is done on host (sum of 4 partials per batch).

v3 design (cost-model driven, evolved from v2):
- Projections unchanged from v2: fp8e4m3 hi+lo DoubleRow matmuls, 3 product
  terms, weights host-prescaled by 16.
- ctx is computed in FLIPPED orientation [q-part, feat-free]: per chunk g,
  8 matmuls (4 q-subtiles x 2 heads) with stationary = expT slice [128,128]
  and moving = v chunk [128,65] (64 feats + ones col). Cost-model matmul
  time is out_free x cyc/row, so feat-free (65) halves ctx PE time vs the
  v2 ctx^T orientation (512-free). The ones column (=16, absorbing the fp8
  v scale) lands the softmax denominator in col 64 of each psum tile.
- Softmax normalize becomes per-partition: DVE reciprocal of the den cols
  + broadcast multiply -> ctx_sb [q, feat] (f32r). No gpsimd
  partition_broadcast needed.
- out-proj needs ctx^T, produced by 8 PE transposes [128,128] per q-block
  (identity moving, f32r, 1.5 cyc/row) into fp psum, evac'd to bf16
  ctxT_sb. out-proj: stationary ctxT (bf16), moving wo (bf16, host-cast).
- DMA order puts all K blocks before V blocks: scores (which clock the exp
  stream) never wait on DMA; ctx tolerates v lateness via the expT pool.
- PSUM: s_ps0/1 (scores double-buffer, 2 banks each), cu0/1 (flipped ctx
  accum, 1 bank each, single-buffered across sweeps with the one-start-
  per-bank opener trick: the first matmul of a sweep start=True clears the
  bank's has_written bits; every other tile's first write overwrites-where-
  clear), fp0/1 (proj/transpose/out-proj) = 8 banks exactly.
"""

from contextlib import ExitStack

import numpy as np

B, S, D, H, DK = 2, 2048, 1024, 16, 64
NCORES = 8
HPC = H // (NCORES // B)      # heads per core = 4
R = HPC * DK                  # local feats = 256
NKC = S // 128                # k-chunks per sweep = 16
NQB = 4                       # 512-wide q blocks
VW = 65                       # v chunk width (64 + ones col)
NSW = 8                       # sweeps = NQB * 2 head-pairs
NG = NSW * NKC                # 128 chunk-slots
EXPSCALE = 0.125 / 256.0      # qT,kT hold 16x values

_CACHE = {}
_LAST_IN_MAPS = None


def _build():
    import concourse.mybir as mybir
    import concourse.tile as tile
    from concourse import bacc

    f32 = mybir.dt.float32
    f32r = mybir.dt.float32r
    bf16 = mybir.dt.bfloat16
    f8 = mybir.dt.float8e4
    DR = mybir.MatmulPerfMode.DoubleRow
    Exp = mybir.ActivationFunctionType.Exp
    Add = mybir.AluOpType.add

    nc = bacc.Bacc(
        "TRN2", target_bir_lowering=False, debug=False,
        enable_asserts=True, num_devices=NCORES,
    )

    # hi/lo fp8 planes interleaved at block granularity (3-dim DMA APs):
    # x2 [D, 4blk*(2plane*512)], w2 [D, 2plane*256] -- one DMA per block
    x_d = {}
    for nm in ("Q2", "K2", "V2"):
        x_d[nm] = nc.dram_tensor(nm, [D, 2 * S], f8, kind="ExternalInput").ap()
    w_d = {}
    for nm in ("wq2", "wk2", "wv2"):
        w_d[nm] = nc.dram_tensor(nm, [D, 2 * R], f8, kind="ExternalInput").ap()
    woT_d = nc.dram_tensor("woT", [R, D], bf16, kind="ExternalInput").ap()
    bq_d = nc.dram_tensor("bq16", [R, 1], f32, kind="ExternalInput").ap()
    out_d = nc.dram_tensor("OUT", [S, D], bf16, kind="ExternalOutput").ap()

    with tile.TileContext(nc) as tc, ExitStack() as ctx:
        sb = ctx.enter_context(tc.tile_pool(name="sb", bufs=1))
        xin = ctx.enter_context(tc.tile_pool(name="xin", bufs=8))
        expp = ctx.enter_context(tc.tile_pool(name="expp", bufs=18))
        normp = ctx.enter_context(tc.tile_pool(name="normp", bufs=4))
        osb = ctx.enter_context(tc.tile_pool(name="osb", bufs=3))
        psum = ctx.enter_context(tc.tile_pool(name="psum", bufs=1, space="PSUM"))

        cnt = {"s": 0, "f": 0}

        def s_tile():
            i = cnt["s"]; cnt["s"] += 1
            return psum.tile([128, 1024], f32, name=f"s_ps{i % 2}", tag=f"s_ps{i % 2}")

        def fk_tile(w=512):
            i = cnt["f"]; cnt["f"] += 1
            return psum.tile([128, w], f32, name=f"fp{i % 2}", tag=f"fp{i % 2}")

        fv_tile = fk_tile

        def cu_tile(h):
            # single-buffered per-head ctx accumulators; 4 q-subtiles per
            # bank at 66-word stride (264B, 8B-aligned)
            return psum.tile([128, 512], f32, name=f"cu{h}", tag=f"cu{h}")

        # ---- persistent SBUF ----
        w_sb = {nm: sb.tile([128, 8 * 2 * R], f8, name=f"{nm}_sb") for nm in w_d}
        bq_sb = sb.tile([128, 2], f32)
        wo_sb = [sb.tile([128, D], bf16, name=f"wo_sb{cn}") for cn in range(2)]
        qT_sb = [sb.tile([128, S], bf16, name=f"qT_sb{hp}") for hp in range(2)]
        kT_sb = [sb.tile([128, S], bf16, name=f"kT_sb{hp}") for hp in range(2)]
        v_all = sb.tile([128, HPC * NKC * VW], bf16)
        # per q-block normalized ctx [q, feat] and its transpose [feat, q]
        ctx_sb = [sb.tile([128, 1024], bf16, name=f"ctx_sb{qvb}")
                  for qvb in range(NQB)]
        ctxT_sb = [[sb.tile([128, 512], bf16, name=f"ctxT{qvb}_{cn}")
                    for cn in range(2)] for qvb in range(NQB)]

        onecol = sb.tile([128, 1], f32)
        nc.vector.memset(onecol[:], 16.0)   # absorbs the 1/16 of fp8 v scale
        vv = v_all.rearrange("p (n c) -> p n c", c=VW)[:, :, 64:65].rearrange(
            "p n c -> p (n c)")
        nc.vector.tensor_copy(vv, onecol[:].broadcast_to((128, HPC * NKC)))

        # ---- DMA: single queue, strict deadline order (the DMA engines
        # device serializes transfers globally at ~2.9us per merged block) ----
        def w_load(eng, nm):
            eng.dma_start(
                w_sb[nm].rearrange("p (d r2) -> p d r2", d=8),
                w_d[nm].rearrange("(d p) r2 -> p d r2", p=128))

        # x block tiles: [128, 8 dchunks, 2*512 (H cols | L cols)] fp8
        xtiles = {}

        def x_load(eng, nm, blk):
            t = xin.tile([128, 8, 1024], f8, name="xin", tag="xin")
            eng.dma_start(
                t[:],
                x_d[nm].rearrange("(d p) c -> p d c", p=128)[
                    :, :, 1024 * blk:1024 * (blk + 1)])
            xtiles[(nm, blk)] = t

        # block 0 of K/Q split into H-then-L plane halves: the H half lands
        # ~3us earlier and feeds 8 of the 12 projection matmuls, pulling the
        # first scores/exp forward
        def x_load_half(eng, nm, half):
            if half == 0:
                t = xin.tile([128, 8, 1024], f8, name="xin", tag="xin")
                xtiles[(nm, 0)] = t
            t = xtiles[(nm, 0)]
            eng.dma_start(
                t[:, :, 512 * half:512 * (half + 1)],
                x_d[nm].rearrange("(d p) c -> p d c", p=128)[
                    :, :, 512 * half:512 * (half + 1)])

        for hp in range(2):
            nc.sync.dma_start(bq_sb[:, hp:hp + 1], bq_d[128 * hp:128 * (hp + 1), :])
        w_load(nc.sync, "wk2")
        w_load(nc.sync, "wq2")
        x_load_half(nc.sync, "K2", 0)
        x_load_half(nc.sync, "Q2", 0)
        x_load_half(nc.sync, "K2", 1)
        x_load_half(nc.sync, "Q2", 1)
        x_load(nc.sync, "K2", 1)
        w_load(nc.sync, "wv2")
        x_load(nc.sync, "K2", 2)
        x_load(nc.sync, "K2", 3)
        for blk in range(4):
            x_load(nc.sync, "V2", blk)
        for cn in range(2):
            nc.sync.dma_start(wo_sb[cn][:], woT_d[128 * cn:128 * (cn + 1), :])
        for blk in range(1, 4):
            x_load(nc.sync, "Q2", blk)

        # ---- fp8 hi/lo DoubleRow projection: 12 matmuls per psum tile ----
        TERMS = (("H", "H"), ("H", "L"), ("L", "H"))

        PL = {"H": 0, "L": 1}

        def proj_mms(p_ps, xnm, blk, wnm, hp, out_sl=slice(0, 512)):
            n = 0
            xt = xtiles[(xnm, blk)]
            wt = w_sb[wnm].rearrange("p (d r2) -> p d r2", d=8)
            for xa, wb in TERMS:
                wo_ = 256 * PL[wb] + 128 * hp
                xo = 512 * PL[xa]
                for j in range(4):
                    nc.tensor.matmul(
                        p_ps[:, out_sl],
                        wt[:, 2 * j:2 * j + 2, wo_:wo_ + 128],
                        xt[:, 2 * j:2 * j + 2, xo:xo + 512],
                        start=(n == 0), stop=(n == 11), perf_mode=DR)
                    n += 1

        def qk_item(xnm, wnm, dst_sb, blk, hp, bias):
            def go():
              from contextlib import nullcontext
              boost = (tc.high_priority(offset=500000) if xnm == "K2"
                       else nullcontext())
              with boost:
                p_ps = fk_tile()
                proj_mms(p_ps, xnm, blk, hp=hp, wnm=wnm)
                dst = dst_sb[hp][:, 512 * blk:512 * (blk + 1)]
                # the evac gates the next sweep's first scores; boost it past
                # v-evacs (500k) but below the mainline (1M)
                with tc.high_priority(offset=750000):
                    if bias:
                        nc.vector.tensor_scalar(
                            dst, p_ps[:], bq_sb[:, hp:hp + 1], None, op0=Add)
                    else:
                        nc.vector.tensor_copy(dst, p_ps[:])
            return go

        def v_item(c):
            def go():
              with tc.high_priority(offset=500000):
                blk, sub = c // 4, c % 4
                v_ps = fv_tile(256)
                n = 0
                xt = xtiles[("V2", blk)]
                wt = w_sb["wv2"].rearrange("p (d r2) -> p d r2", d=8)
                for xa, wb in TERMS:
                    xo = 512 * PL[xa] + 128 * sub
                    wo_ = 256 * PL[wb]
                    for j in range(4):
                        nc.tensor.matmul(
                            v_ps[:, 0:R],
                            xt[:, 2 * j:2 * j + 2, xo:xo + 128],
                            wt[:, 2 * j:2 * j + 2, wo_:wo_ + 256],
                            start=(n == 0), stop=(n == 11), perf_mode=DR)
                        n += 1
                va = v_all.rearrange("p (h n c) -> p h n c", h=HPC, n=NKC)
                nc.vector.tensor_copy(
                    va[:, :, c:c + 1, 0:64],
                    v_ps[:, 0:R].rearrange("p (h n c) -> p h n c", h=HPC, n=1))
              return
            return go

        # ---- attention mainline ----
        def scores(g):
            s, c = divmod(g, NKC)
            qvb, hp = s // 2, s % 2
            s_ps = s_tile()
            for hh in range(2):
                nc.tensor.matmul(
                    s_ps[:, 512 * hh:512 * (hh + 1)],
                    kT_sb[hp][64 * hh:64 * (hh + 1), 128 * c:128 * (c + 1)],
                    qT_sb[hp][64 * hh:64 * (hh + 1), 512 * qvb:512 * (qvb + 1)],
                    start=True, stop=True, skip_group_check=True)
            return s_ps

        # flipped ctx: out [q-part, 65-feat-free]; stationary expT slice,
        # moving v chunk. 4 q-subtiles x 2 heads per chunk g.
        def ctx_g(g, c_ps, expT):
            s, c = divmod(g, NKC)
            hp = s % 2
            for h in range(2):
                gh = 2 * hp + h
                vsl = v_all[:, (gh * NKC + c) * VW:(gh * NKC + c + 1) * VW]
                for qs in range(4):
                    nc.tensor.matmul(
                        c_ps[h][:, 66 * qs:66 * qs + 65],
                        expT[:, 512 * h + 128 * qs:512 * h + 128 * (qs + 1)],
                        vsl,
                        start=(c == 0 and qs == 0),
                        stop=(c == NKC - 1 and qs == 3),
                        skip_group_check=not (c == 0 and qs == 0))

        # ---- norm: per-partition recip + broadcast mul into ctx_sb ----
        def drain_sweep(s, c_ps, last=False):
            hp, qvb = s % 2, s // 2
            with tc.high_priority(offset=1000000):
                for h in range(2):
                    den = c_ps[h][:, 0:264].rearrange(
                        "p (n c) -> p n c", c=66)[:, :, 64:65].rearrange(
                        "p n c -> p (n c)")
                    rb = normp.tile([128, 4], f32, name="rb")
                    nc.vector.reciprocal_approx_fast(out=rb[:], in_=den)
                    hd = 2 * hp + h
                    for qs in range(4):
                        dst = ctx_sb[qvb][:, 256 * qs + 64 * hd:
                                          256 * qs + 64 * hd + 64]
                        src = c_ps[h][:, 66 * qs:66 * qs + 64]
                        if last and h == 1:
                            # tail: ACT is done with exps -- share the drain
                            nc.scalar.mul(dst, src, rb[:, qs:qs + 1])
                        else:
                            nc.vector.tensor_scalar_mul(dst, src, rb[:, qs:qs + 1])

        # ---- transpose ctx [q,f] -> ctxT [f,q] via the DMA xbar (16-bit
        # SBUF->SBUF transpose; the DMA engines are mostly idle mid-stream,
        # and this keeps the transposes entirely off PE/DVE) ----
        def transpose_qvb(qvb, last=False):
            for cn in range(2):
                for qs in range(4):
                    nc.sync.dma_start_transpose(
                        ctxT_sb[qvb][cn][:, 128 * qs:128 * (qs + 1)],
                        ctx_sb[qvb][:, 256 * qs + 128 * cn:
                                    256 * qs + 128 * (cn + 1)])

        # ---- out-proj pieces: (sc, nb) -> partial rows to OUT ----
        o_sbs = {}

        def op_item(sc, nb, tail=False, o_ps=None):
            def go():
                qvb, qs = sc // 4, sc % 4
                if nb == 0:
                    o_sbs[sc] = osb.tile([128, D], bf16, name="o_sb")
                ps = o_ps if o_ps is not None else fv_tile()
                for cn in range(2):
                    nc.tensor.matmul(
                        ps[:],
                        ctxT_sb[qvb][cn][:, 128 * qs:128 * (qs + 1)],
                        wo_sb[cn][:, 512 * nb:512 * (nb + 1)],
                        start=(cn == 0), stop=(cn == 1))
                dst = o_sbs[sc][:, 512 * nb:512 * (nb + 1)]
                if tail and (sc + nb) % 2 == 0:
                    nc.scalar.copy(dst, ps[:])
                else:
                    nc.vector.tensor_copy(dst, ps[:])
                # SWDGE (gpsimd): its drain guarantees completion before
                # program end; full 2KB rows -> max descriptor size
                if nb == 1:
                    nc.gpsimd.dma_start(
                        out_d[128 * sc:128 * (sc + 1), :], o_sbs.pop(sc)[:])
            return go

        # ---- injection schedule (slot g; t(g) ~ 14.4 + 1.04g us, keyed to
        # the measured serial DMA arrival times + sem/margin) ----
        inj = [[] for _ in range(NG + 1)]
        inj[2] += [qk_item("K2", "wk2", kT_sb, 1, 0, False)]
        inj[3] += [qk_item("K2", "wk2", kT_sb, 0, 1, False)]
        inj[4] += [qk_item("Q2", "wq2", qT_sb, 0, 1, True)]
        inj[5] += [qk_item("K2", "wk2", kT_sb, 1, 1, False)]
        inj[6] += [qk_item("K2", "wk2", kT_sb, 2, 0, False)]
        inj[7] += [qk_item("K2", "wk2", kT_sb, 2, 1, False)]
        inj[9] += [qk_item("K2", "wk2", kT_sb, 3, 0, False)]
        inj[10] += [qk_item("K2", "wk2", kT_sb, 3, 1, False)]
        inj[24] += [qk_item("Q2", "wq2", qT_sb, 1, 0, True)]
        inj[32] += [qk_item("Q2", "wq2", qT_sb, 1, 1, True)]
        inj[40] += [qk_item("Q2", "wq2", qT_sb, 2, 0, True)]
        inj[48] += [qk_item("Q2", "wq2", qT_sb, 2, 1, True)]
        inj[56] += [qk_item("Q2", "wq2", qT_sb, 3, 0, True)]
        inj[63] += [qk_item("Q2", "wq2", qT_sb, 3, 1, True)]
        # v chunks at DMA-arrival slots, one per slot; ctx emission is gated
        vemit = {0: 12, 1: 13, 2: 13, 3: 14, 4: 14, 5: 15, 6: 16, 7: 16,
                 8: 17, 9: 18, 10: 19, 11: 19, 12: 20, 13: 21, 14: 22, 15: 22}
        for c, sl in vemit.items():
            inj[sl] += [v_item(c)]

        # ctx(g) may be emitted once v chunks for g are emitted (sweeps 0/1)
        def ctx_gate(g):
            if g >= 2 * NKC:
                return 0
            return vemit[g % NKC] + 1

        # PE warmup: keep the tensor engine busy through the DMA prologue so
        # the pstate ramp completes before real work (cold PE runs 2-4x slow)
        scratch = sb.tile([128, 512], bf16, name="scratch")
        nc.vector.memset(scratch[:], 0.0)

        def pe_dummy(n, tile_fn=None):
            for _ in range(n):
                d_ps = (tile_fn or fk_tile)()
                nc.tensor.matmul(
                    d_ps[:, 0:512], scratch[:, 0:128], scratch[:],
                    start=True, stop=True)

        # enough dummies to keep PE continuously busy until K0-H lands
        # (~6.7us) so the pstate ramp completes and prologue projections run
        # at full clock; pure filler for otherwise-idle PE time
        pe_dummy(22)

        # prologue PE work
        qk_item("K2", "wk2", kT_sb, 0, 0, False)()
        qk_item("Q2", "wq2", qT_sb, 0, 0, True)()

        with tc.high_priority(offset=1000000):
            sps_live = {0: scores(0)}
        expT_live = {}
        cps_live = {}
        op_queue = []  # (sc, nb) pieces whose ctxT is ready, FIFO
        ctx_done = 0  # next ctx chunk-slot to emit

        def emit_ctx_upto(slot, budget=2, no_drain=False):
            nonlocal ctx_done
            while (budget > 0 and ctx_done < slot
                   and ctx_done in expT_live and ctx_gate(ctx_done) < slot):
                g = ctx_done
                s, c = divmod(g, NKC)
                if c == 0:
                    cps_live[s] = [cu_tile(0), cu_tile(1)]
                with tc.high_priority(offset=1000000):
                    ctx_g(g, cps_live[s], expT_live.pop(g)[:])
                if c == NKC - 1 and not (no_drain and s == NSW - 1):
                    c_ps = cps_live.pop(s)
                    drain_sweep(s, c_ps)
                    if s % 2 == 1:
                        transpose_qvb(s // 2)
                        op_queue.extend(
                            (4 * (s // 2) + j // 2, j % 2) for j in range(8))
                ctx_done += 1
                budget -= 1

        for g in range(NG):
            with tc.high_priority(offset=1000000):
                expT = expp.tile([128, 1024], bf16, name="expT")
                nc.scalar.activation(
                    expT[:], sps_live.pop(g)[:], Exp, scale=EXPSCALE)
            expT_live[g] = expT
            # scores(g+1) emitted before ctx: emission order is scheduler
            # priority, and the exp stream is clocked by scores
            if g < NG - 1:
                with tc.high_priority(offset=1000000):
                    sps_live[g + 1] = scores(g + 1)
            emit_ctx_upto(g, budget=1 if g < 32 else (2 if g < 64 else 3))
            for it in inj[g]:
                it()
            # out-proj pieces only in the light late phase: earlier slots are
            # PE-saturated with projections + ctx catch-up, and excess slot
            # work delays the next scores in the in-order PE queue. Hold the
            # last 6 ready pieces in reserve: they fill the tail's PE-idle
            # window while qvb3's norm + dma-transposes are in flight.
            if len(op_queue) > 6 and g >= 64:
                op_item(*op_queue.pop(0))()

        # tail: finish remaining ctx + final drain/transpose, dummies to
        # hold the PE clock through the norm latency, then out-proj qvb=3
        # last ctx chunks WITHOUT the sweep-7 drain: the reserved (ready)
        # out-proj pieces must be emitted before qvb3's dma-transposes, or
        # the conservative monotonic DMA-queue semaphore makes their weight
        # loads falsely wait on those transposes
        while ctx_done < NG:
            emit_ctx_upto(NG + 1, budget=NG, no_drain=True)
        with tc.high_priority(offset=600000):
            while op_queue:
                op_item(*op_queue.pop(0))()
        c_ps = cps_live.pop(NSW - 1)
        drain_sweep(NSW - 1, c_ps, last=True)
        transpose_qvb(3, last=True)
        for j in range(8):
            op_item(12 + j // 2, j % 2, tail=True)()

    nc.compile()
    return nc


def _hl(x):
    import ml_dtypes
    e4 = ml_dtypes.float8_e4m3
    hi = np.asarray(x, e4)
    lo = np.asarray(x - hi.astype(np.float32), e4)
    return hi, lo


def kernel(Q, K, V, wq, bq, wk, bk, wv, bv, wo, bo):
    import ml_dtypes
    from concourse.bass_utils import run_bass_kernel_spmd

    if "nc" not in _CACHE:
        _CACHE["nc"] = _build()
    nc = _CACHE["nc"]

    Q = np.asarray(Q, np.float32)
    K = np.asarray(K, np.float32)
    V = np.asarray(V, np.float32)
    xh = {}
    for nm, t in (("Q2", Q), ("K2", K), ("V2", V)):
        for b in range(B):
            hi, lo = _hl(t[b].T)
            packed = np.stack(
                [hi.reshape(D, 4, 512), lo.reshape(D, 4, 512)], axis=2)
            xh[(nm, b)] = np.ascontiguousarray(packed.reshape(D, 4096))
    wh = {}
    for nm, w in (("wq2", wq), ("wk2", wk), ("wv2", wv)):
        w = np.asarray(w, np.float32)
        for g in range(4):
            hi, lo = _hl(16.0 * w[g * R:(g + 1) * R].T)
            wh[(nm, g)] = np.ascontiguousarray(np.concatenate([hi, lo], axis=1))
    woT = [np.ascontiguousarray(np.asarray(
        np.asarray(wo, np.float32)[:, g * R:(g + 1) * R].T, ml_dtypes.bfloat16))
        for g in range(4)]
    bqs = [np.ascontiguousarray(
        16.0 * np.asarray(bq, np.float32)[g * R:(g + 1) * R, None])
        for g in range(4)]

    in_maps = []
    for c in range(NCORES):
        b, g = c // 4, c % 4
        m = {"woT": woT[g], "bq16": bqs[g]}
        for nm in ("Q2", "K2", "V2"):
            m[nm] = xh[(nm, b)]
        for nm in ("wq2", "wk2", "wv2"):
            m[nm] = wh[(nm, g)]
        in_maps.append(m)

    global _LAST_IN_MAPS
    _LAST_IN_MAPS = in_maps
    res = run_bass_kernel_spmd(nc, in_maps, core_ids=list(range(NCORES)))

    host_bias = (np.asarray(bv, np.float32) @ np.asarray(wo, np.float32).T
                 + np.asarray(bo, np.float32))
    out = np.zeros((B, S, D), np.float32)
    for c in range(NCORES):
        out[c // 4] += np.asarray(res.results[c]["OUT"], np.float32)
    out += host_bias[None, None, :]
    return out
